# revision 1
# baseline (speedup 1.0000x reference)
"""Trainium2 Bass kernel for nn_KANModel (KAN recommender).

Math: with a shared uniform grid (G=5, k=3), the cubic B-spline bases on
the extended uniform knots are shifted cardinal splines:
    B_c(x) = M3(u - c),  u = (x - t0)/h,
    M3(s)  = (1/6) * sum_{m=0..4} (-1)^m C(4,m) relu(s - m)^3.
Folding that combination and the per-edge scales into the layer weights on
the host turns each KAN layer into: z_n = relu(u-n)^3 maps (n = 0..11) plus
one accumulated PE matmul (silu path and bias ride the same accumulation).
n-blocks whose relu is identically zero (from exact host-side range bounds
on the embedding tables / layer-0 output interval) are dropped entirely.
silu(x) is computed as x*sigmoid(x) so every activation used (Sigmoid,
Square) lives in one ACT table set -> a single table load.
Data-parallel over batch: 1024 rows -> 8 cores x 128.
"""

import numpy as np

B_FULL = 1024
NCORES = 8
BS = B_FULL // NCORES          # batch shard per core
D = 64                         # embedding dim
IN0, OUT0 = 2 * D, 64          # KAN layer 0
IN1 = 64                       # KAN layer 1 (out_dim 1)
G, KORD = 5, 3
NC_BASIS = G + KORD            # 8 spline bases per edge
NZ = G + 2 * KORD + 1          # 12 possible relu-cube shifts
NU, NI = 100000, 50000

_BUILD_CACHE = {}
TRACE = False
LAST_RESULTS = None

_A5 = np.array([1.0, -4.0, 6.0, -4.0, 1.0], dtype=np.float64) / 6.0


def _m3(s):
    """Cardinal cubic B-spline, exact (clamped) evaluation, float64."""
    s = np.minimum(s, 4.0)
    out = np.zeros_like(s)
    for m in range(4):
        r = np.maximum(s - m, 0.0)
        out += _A5[m] * r * r * r
    return out


def _fold_host_weights(grid0, coef0, sb0, ssp0, bias0, grid1, coef1, sb1, ssp1,
                       bias1, x_min, x_max):
    """O(params) host-side prep: folded weights, layouts, and exact/rigorous
    n-block ranges for both layers."""
    h0 = float(grid0[0, -1] - grid0[0, 0]) / G
    t0_0 = float(grid0[0, 0]) - KORD * h0
    h1 = float(grid1[0, -1] - grid1[0, 0]) / G
    t0_1 = float(grid1[0, 0]) - KORD * h1

    # ---- layer-0 n-trim: exact from table extrema ----
    u0_max = (x_max - t0_0) / h0
    nlist0 = [n for n in range(NZ) if n < u0_max + 1e-6]

    c0e = (ssp0[:, None].astype(np.float64) * coef0.astype(np.float64)).reshape(
        OUT0, IN0, NC_BASIS
    )  # (o, f, c)
    wz0 = np.zeros((len(nlist0), IN0, OUT0), dtype=np.float64)
    for k, n in enumerate(nlist0):
        for m in range(5):
            c = n - m
            if 0 <= c < NC_BASIS:
                wz0[k] += _A5[m] * c0e[:, :, c].T
    wz0_sb = np.ascontiguousarray(
        wz0.transpose(1, 0, 2).reshape(IN0, len(nlist0) * OUT0).astype(np.float32)
    )  # [f, k*OUT0+o]
    sb0e = sb0.reshape(OUT0, IN0).astype(np.float64)  # (o, f)
    sb0_sb = np.ascontiguousarray(sb0e.T.astype(np.float32))
    bias0_sb = np.ascontiguousarray(bias0.reshape(1, OUT0).astype(np.float32))

    # ---- rigorous layer-0 output interval (grid + Lipschitz pad) ----
    NGRID = 2049
    xg = np.linspace(x_min, x_max, NGRID)
    dx = (x_max - x_min) / (NGRID - 1) if x_max > x_min else 0.0
    ug = (xg - t0_0) / h0
    basis = np.stack([_m3(ug - c) for c in range(NC_BASIS)], axis=1)  # (g, c)
    silug = xg / (1.0 + np.exp(-xg))
    # edge values phi[o,f,g] = sb*silu + sum_c c0e*basis
    phi = sb0e[:, :, None] * silug[None, None, :] + np.einsum(
        "ofc,gc->ofg", c0e, basis
    )
    # Lipschitz bound per edge: |phi'| <= |sb|*1.1 + sum_c |c0e_c| * 0.75/h0
    lip = np.abs(sb0e) * 1.1 + np.abs(c0e).sum(axis=2) * (0.75 / h0)
    pad = lip * dx
    h_min = bias0.astype(np.float64) + (phi.min(axis=2) - pad).sum(axis=1)
    h_max = bias0.astype(np.float64) + (phi.max(axis=2) + pad).sum(axis=1)
    u1_max = (float(h_max.max()) - t0_1) / h1
    nlist1 = [n for n in range(NZ) if n < u1_max + 1e-3]

    # ---- layer-1 folded weights ----
    c1e = ssp1[:, None].astype(np.float64) * coef1.astype(np.float64)  # (64, 8)
    wz1 = np.zeros((len(nlist1), IN1), dtype=np.float64)
    for k, n in enumerate(nlist1):
        for m in range(5):
            c = n - m
            if 0 <= c < NC_BASIS:
                wz1[k] += _A5[m] * c1e[:, c]
    zlen = len(nlist1) * IN1
    w1flat = np.concatenate(
        [wz1.reshape(-1), sb1.astype(np.float64)]
    ).astype(np.float32)
    w1big = np.ascontiguousarray(np.broadcast_to(w1flat, (128, zlen + IN1)).copy())

    consts = (
        t0_0, 1.0 / h0, t0_1, 1.0 / h1, float(bias1[0]),
        tuple(nlist0), tuple(nlist1),
    )
    return consts, dict(wz0=wz0_sb, sb0w=sb0_sb, bias0r=bias0_sb, w1big=w1big)


def _build_program(consts):
    import concourse.bass as bass
    import concourse.bacc as bacc
    import concourse.mybir as mybir
    from concourse.tile import TileContext
    from concourse.masks import make_identity

    t0_0, inv_h0, t0_1, inv_h1, bias1, nlist0, nlist1 = consts
    L0, L1 = len(nlist0), len(nlist1)
    ZL = L1 * IN1               # layer-1 z-block width
    WL = ZL + IN1               # plus silu block
    f32 = mybir.dt.float32
    i32 = mybir.dt.int32
    A = mybir.AluOpType
    AF = mybir.ActivationFunctionType

    nc = bacc.Bacc("TRN2")
    d_idx = nc.dram_tensor("idx", [BS, 2], i32, kind="ExternalInput")
    d_eu = nc.dram_tensor("emb_user", [NU, D], f32, kind="ExternalInput")
    d_ei = nc.dram_tensor("emb_item", [NI, D], f32, kind="ExternalInput")
    d_wz0 = nc.dram_tensor("wz0", [IN0, L0 * OUT0], f32, kind="ExternalInput")
    d_sb0 = nc.dram_tensor("sb0w", [IN0, OUT0], f32, kind="ExternalInput")
    d_b0 = nc.dram_tensor("bias0r", [1, OUT0], f32, kind="ExternalInput")
    d_w1 = nc.dram_tensor("w1big", [128, WL], f32, kind="ExternalInput")
    d_out = nc.dram_tensor("out", [BS, 1], f32, kind="ExternalOutput")

    with TileContext(nc) as tc:
        with (
            tc.tile_pool(name="sb", bufs=1) as P,
            tc.tile_pool(name="ps", bufs=1, space="PSUM") as PS,
        ):
            idx = P.tile([BS, 2], i32, tag="idx")
            nc.sync.dma_start(out=idx[:], in_=d_idx[:])
            ident = P.tile([128, 128], f32, tag="ident")
            make_identity(nc, ident[:])
            wz0 = P.tile([IN0, L0 * OUT0], f32, tag="wz0")
            nc.sync.dma_start(out=wz0[:], in_=d_wz0[:])
            sb0 = P.tile([IN0, OUT0], f32, tag="sb0")
            nc.sync.dma_start(out=sb0[:], in_=d_sb0[:])
            b0 = P.tile([1, OUT0], f32, tag="b0")
            nc.sync.dma_start(out=b0[:1, :], in_=d_b0[:])
            w1 = P.tile([128, WL], f32, tag="w1")
            nc.sync.dma_start(out=w1[:], in_=d_w1[:])
            ones = P.tile([1, 128], f32, tag="ones")
            nc.gpsimd.memset(ones[:1, :], 1.0)

            # gather embeddings: row b of each table -> partition b
            xbm = P.tile([BS, 2 * D], f32, tag="xbm")
            nc.gpsimd.indirect_dma_start(
                out=xbm[:, 0:D], out_offset=None, in_=d_eu[:],
                in_offset=bass.IndirectOffsetOnAxis(ap=idx[:, 0:1], axis=0),
            )
            nc.gpsimd.indirect_dma_start(
                out=xbm[:, D : 2 * D], out_offset=None, in_=d_ei[:],
                in_offset=bass.IndirectOffsetOnAxis(ap=idx[:, 1:2], axis=0),
            )

            # transpose to feature-major x^T: (f, b)
            xT = PS.tile([128, BS], f32, tag="xT")
            nc.tensor.matmul(out=xT[:], lhsT=xbm[:], rhs=ident[:],
                             is_transpose=True, start=True, stop=True)

            u0 = P.tile([128, BS], f32, tag="u0")
            nc.vector.tensor_scalar(u0[:], xT[:], t0_0, inv_h0, A.subtract, A.mult)
            # silu(x) = x * sigmoid(x): keeps ACT funcs inside one table set
            sg0 = P.tile([128, BS], f32, tag="sg0")
            nc.scalar.activation(sg0[:], xT[:], AF.Sigmoid)
            silu0 = P.tile([128, BS], f32, tag="silu0")
            nc.vector.tensor_tensor(out=silu0[:], in0=sg0[:], in1=xT[:], op=A.mult)

            # layer-0 accumulation in PSUM: h[b, o]
            hps = PS.tile([BS, OUT0], f32, tag="hps")
            nc.tensor.matmul(out=hps[:], lhsT=ones[:1, :], rhs=b0[:1, :],
                             start=True, stop=False)
            nc.tensor.matmul(out=hps[:], lhsT=silu0[:], rhs=sb0[:],
                             start=False, stop=False)

            rbig = P.tile([128, L0 * BS], f32, tag="rbig")
            qbig = P.tile([128, L0 * BS], f32, tag="qbig")
            zbig = P.tile([128, L0 * BS], f32, tag="zbig")
            CH = 2  # n-values per r/q/z pipeline chunk
            chunks = [list(range(s, min(s + CH, L0))) for s in range(0, L0, CH)]
            for ci, ch in enumerate(chunks):
                for j, k in enumerate(ch):
                    n = nlist0[k]
                    eng = nc.vector if j % 2 == 0 else nc.gpsimd
                    eng.tensor_scalar(
                        rbig[:, k * BS : (k + 1) * BS], u0[:],
                        float(n), 0.0, A.subtract, A.max,
                    )
                sl = slice(ch[0] * BS, (ch[-1] + 1) * BS)
                nc.scalar.activation(qbig[:, sl], rbig[:, sl], AF.Square)
                nc.vector.tensor_tensor(out=zbig[:, sl], in0=qbig[:, sl],
                                        in1=rbig[:, sl], op=A.mult)
                for k in ch:
                    nc.tensor.matmul(
                        out=hps[:],
                        lhsT=zbig[:, k * BS : (k + 1) * BS],
                        rhs=wz0[:, k * OUT0 : (k + 1) * OUT0],
                        start=False, stop=(k == L0 - 1),
                    )

            # ---- layer 1 (free-axis contraction) ----
            u1 = P.tile([BS, IN1], f32, tag="u1")
            nc.vector.tensor_scalar(u1[:], hps[:], t0_1, inv_h1, A.subtract, A.mult)
            # right operand: [q1 blocks | h*sb1], left operand: [r1*w1z | sigmoid(h)]
            left = P.tile([BS, WL], f32, tag="left")
            right = P.tile([BS, WL], f32, tag="right")
            nc.scalar.activation(left[:, ZL:WL], hps[:], AF.Sigmoid)
            nc.vector.tensor_tensor(out=right[:, ZL:WL], in0=hps[:], in1=w1[:, ZL:WL],
                                    op=A.mult)

            r1 = P.tile([BS, ZL], f32, tag="r1")
            for k, n in enumerate(nlist1):
                eng = nc.vector if k % 2 == 0 else nc.gpsimd
                eng.tensor_scalar(
                    r1[:, k * IN1 : (k + 1) * IN1], u1[:],
                    float(n), 0.0, A.subtract, A.max,
                )
            nc.scalar.activation(right[:, 0:ZL], r1[:], AF.Square)
            nc.vector.tensor_tensor(out=left[:, 0:ZL], in0=r1[:], in1=w1[:, 0:ZL],
                                    op=A.mult)

            prod = P.tile([BS, WL], f32, tag="prod")
            nc.vector.tensor_tensor(out=prod[:], in0=left[:], in1=right[:],
                                    op=A.mult)
            y = P.tile([BS, 1], f32, tag="y")
            nc.vector.tensor_reduce(y[:], prod[:], axis=mybir.AxisListType.X,
                                    op=A.add)
            osb = P.tile([BS, 1], f32, tag="osb")
            nc.scalar.activation(osb[:], y[:], AF.Sigmoid, bias=float(bias1))
            nc.sync.dma_start(out=d_out[:], in_=osb[:])

    nc.compile()
    return nc


def kernel(
    user_indices, item_indices, grid_update_num, stop_grid_update_step,
    emb_user, emb_item,
    grid0, coef0, sb0, ssp0, bias0,
    grid1, coef1, sb1, ssp1, bias1,
):
    global LAST_RESULTS
    from concourse.bass_utils import run_bass_kernel_spmd

    uidx = np.asarray(user_indices).astype(np.int32).reshape(B_FULL, 1)
    iidx = np.asarray(item_indices).astype(np.int32).reshape(B_FULL, 1)
    eu = np.ascontiguousarray(np.asarray(emb_user, dtype=np.float32))
    ei = np.ascontiguousarray(np.asarray(emb_item, dtype=np.float32))
    x_min = float(min(eu.min(), ei.min()))
    x_max = float(max(eu.max(), ei.max()))

    consts, w = _fold_host_weights(
        np.asarray(grid0, dtype=np.float32), np.asarray(coef0, dtype=np.float32),
        np.asarray(sb0, dtype=np.float32), np.asarray(ssp0, dtype=np.float32),
        np.asarray(bias0, dtype=np.float32), np.asarray(grid1, dtype=np.float32),
        np.asarray(coef1, dtype=np.float32), np.asarray(sb1, dtype=np.float32),
        np.asarray(ssp1, dtype=np.float32), np.asarray(bias1, dtype=np.float32),
        x_min, x_max,
    )

    if consts not in _BUILD_CACHE:
        _BUILD_CACHE[consts] = _build_program(consts)
    nc = _BUILD_CACHE[consts]

    in_maps = []
    for c in range(NCORES):
        sl = slice(c * BS, (c + 1) * BS)
        in_maps.append(
            {
                "idx": np.ascontiguousarray(
                    np.concatenate([uidx[sl], iidx[sl]], axis=1)),
                "emb_user": eu,
                "emb_item": ei,
                "wz0": w["wz0"],
                "sb0w": w["sb0w"],
                "bias0r": w["bias0r"],
                "w1big": w["w1big"],
            }
        )

    res = run_bass_kernel_spmd(nc, in_maps, core_ids=list(range(NCORES)),
                               trace=TRACE)
    LAST_RESULTS = res
    return np.concatenate([r["out"] for r in res.results], axis=0)

